# revision 30
# baseline (speedup 1.0000x reference)
"""Trainium2 Bass kernel for nn_Attention_8366596292664.

Dense transformer block: qkv proj -> RoPE -> GQA causal attention ->
out proj -> RMSNorm.  B=4, S=2048, H=2048, 16 heads (hd=128), 4 KV heads.

Sharding: 8 cores = (4 batches) x (2 interleaved query-row parities).
Core (b, par) computes the full block for query rows {par, par+2, ...} of
batch b.  Interleaving the query rows by parity makes the causal structure
identical on every core, so one SPMD program serves all 8 cores; the
parity enters only through the data (a 1-column roll of x^T, cos/sin
tables, and the output row scatter).

Layout strategy (all matmuls contract over the partition dim):
  - x^T   [h, s]   : host-transposed, bf16
  - qkv^T [f, s]   : produced directly by the projection (W rows = contraction)
  - RoPE applied in transposed layout; the even/odd pair interleave is
    converted to a halves layout by permuting W_q / W_k columns on host.
  - scores^T [k, q]: k-tile stationary, q moving -> softmax runs along
    partitions via a DVE accumulation tree + ones-matmul (no transposes).
  - y^T  [d, q]    : v natural-layout stationary, probs^T moving.
  - proj           : y^T slices stationary, W_proj natural moving; output
    lands in [s, o] layout where RMSNorm is a free-dim reduction.
"""

import numpy as np
import ml_dtypes

BF16 = ml_dtypes.bfloat16

# ---------------------------------------------------------------- config
P = 128          # partitions
HD = 128         # head dim
HH = HD // 2     # rope half
G = 4            # GQA group size

B = 4
S = 2048
H = 2048
N_CORES = 8

NH = H // HD          # 16 q heads
NKV = NH // G         # 4 kv heads
KVC = NKV * HD        # 512 kv columns
HT = H // P           # 16 h-tiles (contraction tiles)
S_LOC = S // 2        # 1024 local q rows per core
IT = 512              # i-tile (queries per score tile, = 1 psum bank fp32)
NT_I = S_LOC // IT    # 2 i-slots
SPAN = S // NT_I      # 1024 global rows per slot
JB = SPAN // P        # 8 j-tiles in the diagonal band of each slot
OT = 512              # output-proj column tile
NO = H // OT          # 4

RMS_EPS = 1e-6
SCALE = 1.0 / float(np.sqrt(np.float32(HD)))

_CACHE = {}


# ---------------------------------------------------------------- device IR
def _build_nc():
    from contextlib import ExitStack

    import concourse.bacc as bacc
    import concourse.mybir as mybir
    import concourse.tile as tile

    dt = mybir.dt
    AF = mybir.ActivationFunctionType

    nc = bacc.Bacc("TRN2", target_bir_lowering=False, debug=False)

    xt_d = nc.dram_tensor("xt", [HT, P, S], dt.bfloat16, kind="ExternalInput")
    wq_d = nc.dram_tensor("wq", [NH, P, HT, HD], dt.bfloat16, kind="ExternalInput")
    wk_d = nc.dram_tensor("wk", [NKV, P, HT, HD], dt.bfloat16, kind="ExternalInput")
    wv_d = nc.dram_tensor("wv", [HT, P, KVC], dt.bfloat16, kind="ExternalInput")
    wp_d = nc.dram_tensor("wp", [HT, P, H], dt.bfloat16, kind="ExternalInput")
    qcos_d = nc.dram_tensor("qcos", [HH, S_LOC], dt.float32, kind="ExternalInput")
    qsin_d = nc.dram_tensor("qsin", [HH, S_LOC], dt.float32, kind="ExternalInput")
    kcos_d = nc.dram_tensor("kcos", [HH, S], dt.float32, kind="ExternalInput")
    ksin_d = nc.dram_tensor("ksin", [HH, S], dt.float32, kind="ExternalInput")
    mask_d = nc.dram_tensor("mask", [JB, P, IT], dt.bfloat16, kind="ExternalInput")
    nw_d = nc.dram_tensor("nw", [P, H], dt.float32, kind="ExternalInput")
    out_d = nc.dram_tensor("out", [S_LOC, H], dt.float32, kind="ExternalOutput")

    with tile.TileContext(nc) as tc, ExitStack() as body:
        const = body.enter_context(tc.tile_pool(name="const", bufs=1))
        qcos = const.tile([HH, S_LOC], dt.float32)
        qsin = const.tile([HH, S_LOC], dt.float32)
        kcos = const.tile([HH, S], dt.float32)
        ksin = const.tile([HH, S], dt.float32)
        ones = const.tile([P, 1], dt.float32)
        epsb = const.tile([P, 1], dt.float32)
        nc.vector.memset(epsb[:], RMS_EPS)
        nc.sync.dma_start(qcos[:], qcos_d.ap())
        nc.sync.dma_start(qsin[:], qsin_d.ap())
        nc.sync.dma_start(kcos[:], kcos_d.ap())
        nc.sync.dma_start(ksin[:], ksin_d.ap())
        nc.vector.memset(ones[:], 1.0)

        s_act = body.enter_context(ExitStack())
        act = s_act.enter_context(tc.tile_pool(name="act", bufs=1))
        qT = act.tile([P, NH * S_LOC], dt.bfloat16)
        kT = act.tile([P, NKV * S], dt.bfloat16)
        vv = act.tile([P, (S // P) * KVC], dt.bfloat16)

        def rope_evict(rpool, ps, dst_lo, dst_hi, cs, sn):
            # dst_lo = ps_lo*cos - ps_hi*sin ; dst_hi = ps_hi*cos + ps_lo*sin
            t1 = rpool.tile([HH, IT], dt.float32, name="rt1")
            t2 = rpool.tile([HH, IT], dt.float32, name="rt2")
            nc.vector.tensor_mul(t1[:], ps[0:HH, :], cs)
            nc.vector.tensor_mul(t2[:], ps[HH:P, :], sn)
            nc.vector.tensor_sub(dst_lo, t1[:], t2[:])
            nc.vector.tensor_mul(t1[:], ps[HH:P, :], cs)
            nc.vector.tensor_mul(t2[:], ps[0:HH, :], sn)
            nc.vector.tensor_add(dst_hi, t1[:], t2[:])

        # ---------------- phase 1: qkv projection + rope --------------
        with ExitStack() as ph1:
            xp = ph1.enter_context(tc.tile_pool(name="xp", bufs=1))
            xt = xp.tile([P, HT * S], dt.bfloat16)
            nc.sync.dma_start(
                xt[:].rearrange("p (t s) -> p t s", t=HT),
                xt_d.ap().rearrange("t p s -> p t s"),
            )

            wkp = ph1.enter_context(tc.tile_pool(name="wkp", bufs=1))
            wk = wkp.tile([P, NKV * HT * HD], dt.bfloat16)
            nc.sync.dma_start(
                wk[:].rearrange("p (f t m) -> p f t m", f=NKV, t=HT),
                wk_d.ap().rearrange("f p t m -> p f t m"),
            )

            wvp = ph1.enter_context(tc.tile_pool(name="wvp", bufs=1))
            wv = wvp.tile([P, HT * KVC], dt.bfloat16)
            nc.sync.dma_start(
                wv[:].rearrange("p (t f) -> p t f", t=HT),
                wv_d.ap().rearrange("t p f -> p t f"),
            )

            wqp = ph1.enter_context(tc.tile_pool(name="wqp", bufs=2))
            rp1 = ph1.enter_context(tc.tile_pool(name="rp1", bufs=2))
            psq = ph1.enter_context(tc.tile_pool(name="psq", bufs=4, space="PSUM"))
            psk = ph1.enter_context(tc.tile_pool(name="psk", bufs=2, space="PSUM"))
            psv = ph1.enter_context(tc.tile_pool(name="psv", bufs=2, space="PSUM"))

            # k projection (all S columns) + rope
            for fk in range(NKV):
                for sc in range(S // IT):
                    ps = psk.tile([P, IT], dt.float32, name="kps")
                    for h in range(HT):
                        nc.tensor.matmul(
                            ps[:],
                            wk[:, fk * H + h * HD : fk * H + (h + 1) * HD],
                            xt[:, h * S + sc * IT : h * S + (sc + 1) * IT],
                            start=(h == 0),
                            stop=(h == HT - 1),
                        )
                    c0 = fk * S + sc * IT
                    rope_evict(
                        rp1, ps,
                        kT[0:HH, c0 : c0 + IT], kT[HH:P, c0 : c0 + IT],
                        kcos[:, sc * IT : (sc + 1) * IT],
                        ksin[:, sc * IT : (sc + 1) * IT],
                    )

            # q projection (local rows, stride-2 reads of x^T) + rope
            for fq in range(NH):
                wq = wqp.tile([P, HT * HD], dt.bfloat16, name="wqt")
                nc.sync.dma_start(
                    wq[:].rearrange("p (t m) -> p t m", t=HT),
                    wq_d.ap()[fq],
                )
                pss = [psq.tile([P, IT], dt.float32, name="qps") for _ in range(NT_I)]
                for h in range(HT):
                    for t in range(NT_I):
                        st = h * S + 2 * t * IT
                        nc.tensor.matmul(
                            pss[t][:],
                            wq[:, h * HD : (h + 1) * HD],
                            xt[:, st : st + 2 * IT : 2],
                            start=(h == 0),
                            stop=(h == HT - 1),
                        )
                for t in range(NT_I):
                    c0 = fq * S_LOC + t * IT
                    rope_evict(
                        rp1, pss[t],
                        qT[0:HH, c0 : c0 + IT], qT[HH:P, c0 : c0 + IT],
                        qcos[:, t * IT : (t + 1) * IT],
                        qsin[:, t * IT : (t + 1) * IT],
                    )

            # v projection (natural [s, f] layout)
            for sv in range(S // P):
                ps = psv.tile([P, KVC], dt.float32, name="vps")
                for h in range(HT):
                    nc.tensor.matmul(
                        ps[:],
                        xt[:, h * S + sv * P : h * S + (sv + 1) * P],
                        wv[:, h * KVC : (h + 1) * KVC],
                        start=(h == 0),
                        stop=(h == HT - 1),
                    )
                nc.scalar.activation(
                    vv[:, sv * KVC : (sv + 1) * KVC], ps[:], AF.Copy
                )

        # ---------------- phase 2: attention --------------------------
        late = body.enter_context(tc.tile_pool(name="late", bufs=1, side="right"))
        masks = late.tile([P, JB * IT], dt.bfloat16)
        nc.sync.dma_start(
            masks[:].rearrange("p (j f) -> p j f", j=JB),
            mask_d.ap().rearrange("j p f -> p j f"),
        )
        nw = late.tile([P, H], dt.float32)
        nc.sync.dma_start(nw[:], nw_d.ap())
        yT = late.tile([P, NH * S_LOC], dt.bfloat16)

        with ExitStack() as ph2:
            prp = ph2.enter_context(tc.tile_pool(name="prp", bufs=6))
            accp = ph2.enter_context(tc.tile_pool(name="accp", bufs=2))
            recp = ph2.enter_context(tc.tile_pool(name="recp", bufs=2))
            rbcp = ph2.enter_context(tc.tile_pool(name="rbcp", bufs=2))
            pss_p = ph2.enter_context(tc.tile_pool(name="pssp", bufs=4, space="PSUM"))
            psy = ph2.enter_context(tc.tile_pool(name="psy", bufs=2, space="PSUM"))
            psd = ph2.enter_context(tc.tile_pool(name="psd", bufs=1, space="PSUM"))

            for hq in range(NH):
                kvh = hq // G
                for t in range(NT_I):
                    nj = (t + 1) * JB
                    yps = psy.tile([P, IT], dt.float32, name="yps")
                    acc = accp.tile([P, IT], dt.float32, name="acc")
                    for j in range(nj):
                        sps = pss_p.tile([P, IT], dt.float32, name="sps")
                        nc.tensor.matmul(
                            sps[:],
                            kT[:, kvh * S + j * P : kvh * S + (j + 1) * P],
                            qT[:, hq * S_LOC + t * IT : hq * S_LOC + (t + 1) * IT],
                            start=True,
                            stop=True,
                        )
                        pr = prp.tile([P, IT], dt.bfloat16, name="pr")
                        nc.scalar.activation(pr[:], sps[:], AF.Exp, scale=SCALE)
                        jj = j - t * JB
                        if jj >= 0:
                            nc.gpsimd.tensor_mul(
                                pr[:], pr[:], masks[:, jj * IT : (jj + 1) * IT]
                            )
                        if j == 0:
                            nc.vector.tensor_copy(acc[:], pr[:])
                        else:
                            nc.vector.tensor_add(acc[:], acc[:], pr[:])
                        nc.tensor.matmul(
                            yps[:],
                            vv[:, j * KVC + kvh * HD : j * KVC + (kvh + 1) * HD],
                            pr[:],
                            start=(j == 0),
                            stop=(j == nj - 1),
                        )
                    dps = psd.tile([1, IT], dt.float32, name="dps")
                    nc.tensor.matmul(dps[:], ones[:], acc[:], start=True, stop=True)
                    rec = recp.tile([1, IT], dt.float32, name="rec")
                    nc.vector.reciprocal(rec[:], dps[:])
                    rbc = rbcp.tile([P, IT], dt.float32, name="rbc")
                    nc.gpsimd.partition_broadcast(rbc[:], rec[:])
                    nc.vector.tensor_mul(
                        yT[:, hq * S_LOC + t * IT : hq * S_LOC + (t + 1) * IT],
                        yps[:],
                        rbc[:],
                    )

        s_act.close()  # free qT / kT / vv before the projection phase

        # ---------------- phase 3: out projection + rmsnorm ------------
        with ExitStack() as ph3:
            wpp = ph3.enter_context(tc.tile_pool(name="wpp", bufs=1))
            wp = wpp.tile([P, HT * H], dt.bfloat16)
            nc.sync.dma_start(
                wp[:].rearrange("p (t f) -> p t f", t=HT),
                wp_d.ap().rearrange("t p f -> p t f"),
            )
            outp = ph3.enter_context(tc.tile_pool(name="outp", bufs=2))
            sqp = ph3.enter_context(tc.tile_pool(name="sqp", bufs=2))
            smp = ph3.enter_context(tc.tile_pool(name="smp", bufs=2))
            po = ph3.enter_context(tc.tile_pool(name="po", bufs=8, space="PSUM"))

            for sl in range(S_LOC // P):
                pso = [po.tile([P, OT], dt.float32, name="pso") for _ in range(NO)]
                for h in range(HT):
                    lhs = yT[:, h * S_LOC + sl * P : h * S_LOC + (sl + 1) * P]
                    for o in range(NO):
                        nc.tensor.matmul(
                            pso[o][:],
                            lhs,
                            wp[:, h * H + o * OT : h * H + (o + 1) * OT],
                            start=(h == 0),
                            stop=(h == HT - 1),
                        )
                ot = outp.tile([P, H], dt.float32, name="ot")
                for o in range(NO):
                    nc.scalar.activation(
                        ot[:, o * OT : (o + 1) * OT], pso[o][:], AF.Copy
                    )
                sq = sqp.tile([P, H], dt.float32, name="sq")
                ssq = smp.tile([P, 1], dt.float32, name="ssq")
                nc.scalar.activation(sq[:], ot[:], AF.Square, accum_out=ssq[:])
                rms = smp.tile([P, 1], dt.float32, name="rms")
                nc.scalar.activation(
                    rms[:], ssq[:], AF.Sqrt, bias=epsb[:], scale=1.0 / H
                )
                rr = smp.tile([P, 1], dt.float32, name="rr")
                nc.vector.reciprocal(rr[:], rms[:])
                nc.vector.tensor_scalar_mul(ot[:], ot[:], rr[:])
                nc.vector.tensor_mul(ot[:], ot[:], nw[:])
                nc.sync.dma_start(out_d.ap()[sl * P : (sl + 1) * P, :], ot[:])

    nc.compile()
    return nc


# ---------------------------------------------------------------- host side
def _host_shared(w_attn, w_proj, norm_w):
    """Core-independent packed tensors."""
    f32 = np.float32

    def perm_halves(w):  # [H, n, HD] even/odd pairs -> halves
        return np.concatenate([w[..., 0::2], w[..., 1::2]], axis=-1)

    wq = perm_halves(w_attn[:, :H].reshape(H, NH, HD))
    wq = np.ascontiguousarray(
        wq.reshape(HT, P, NH, HD).transpose(2, 1, 0, 3)
    ).astype(BF16)
    wk = perm_halves(w_attn[:, H : H + KVC].reshape(H, NKV, HD))
    wk = np.ascontiguousarray(
        wk.reshape(HT, P, NKV, HD).transpose(2, 1, 0, 3)
    ).astype(BF16)
    wv = np.ascontiguousarray(
        w_attn[:, H + KVC :].reshape(HT, P, KVC)
    ).astype(BF16)
    wp = np.ascontiguousarray(w_proj.reshape(HT, P, H)).astype(BF16)

    jj, p, f = np.meshgrid(
        np.arange(JB), np.arange(P), np.arange(IT), indexing="ij"
    )
    # parity 0: query 2f vs key (128jj + p)
    mask0 = (2 * f >= 128 * jj + p).astype(BF16)
    # parity 1: query 2f+1 vs key (128jj + (p^1))  (pair-swapped x columns)
    mask1 = (2 * f + 1 >= 128 * jj + (p ^ 1)).astype(BF16)

    nw = np.ascontiguousarray(
        np.broadcast_to(norm_w.astype(f32), (P, H))
    )
    return wq, wk, wv, wp, (mask0, mask1), nw


def _cos_sin(pos):
    f32 = np.float32
    inv = 1.0 / (
        10000.0 ** (np.arange(0, HD, 2, dtype=f32) / f32(HD))
    )
    ang = inv[:, None].astype(f32) * pos[None, :].astype(f32)  # [HH, N]
    return np.cos(ang).astype(f32), np.sin(ang).astype(f32)


def make_in_maps(x, w_attn, w_proj, norm_w):
    x = np.asarray(x, dtype=np.float32)
    w_attn = np.asarray(w_attn, dtype=np.float32)
    w_proj = np.asarray(w_proj, dtype=np.float32)
    norm_w = np.asarray(norm_w, dtype=np.float32)

    wq, wk, wv, wp, (mask0, mask1), nw = _host_shared(w_attn, w_proj, norm_w)

    kc0, ks0 = _cos_sin(np.arange(S, dtype=np.float32))          # parity 0
    # parity 1: column j holds global row j^1 (pair-swapped x columns)
    kc1, ks1 = _cos_sin((np.arange(S) ^ 1).astype(np.float32))
    qc0, qs0 = _cos_sin(2.0 * np.arange(S_LOC, dtype=np.float32))
    qc1, qs1 = _cos_sin(2.0 * np.arange(S_LOC, dtype=np.float32) + 1.0)

    in_maps = []
    for c in range(N_CORES):
        b, par = c // 2, c % 2
        xt = x[b].T.astype(BF16)
        if par:
            xt = xt[:, np.arange(S) ^ 1]  # swap adjacent column pairs
        xt = np.ascontiguousarray(xt.reshape(HT, P, S))
        in_maps.append(
            {
                "xt": xt,
                "wq": wq,
                "wk": wk,
                "wv": wv,
                "wp": wp,
                "qcos": qc1 if par else qc0,
                "qsin": qs1 if par else qs0,
                "kcos": kc1 if par else kc0,
                "ksin": ks1 if par else ks0,
                "mask": mask1 if par else mask0,
                "nw": nw,
            }
        )
    return in_maps


def assemble_out(results):
    out = np.empty((B, S, H), dtype=np.float32)
    for c in range(N_CORES):
        b, par = c // 2, c % 2
        out[b, par::2, :] = results[c]["out"]
    return out


def kernel(x, w_attn, w_proj, norm_w):
    from concourse import bass_utils

    if "nc" not in _CACHE:
        _CACHE["nc"] = _build_nc()
    nc = _CACHE["nc"]

    in_maps = make_in_maps(x, w_attn, w_proj, norm_w)
    res = bass_utils.run_bass_kernel_spmd(
        nc, in_maps, core_ids=list(range(N_CORES))
    )
    return assemble_out(res.results)


# revision 33
# speedup vs baseline: 1.7530x; 1.7530x over previous
"""Trainium2 Bass kernel for nn_Attention_8366596292664.

Dense transformer block: qkv proj -> RoPE -> GQA causal attention ->
out proj -> RMSNorm.  B=4, S=2048, H=2048, 16 heads (hd=128), 4 KV heads.

Sharding: 8 cores = (4 batches) x (2 interleaved query-row parities).
Core (b, par) computes the full block for query rows {par, par+2, ...} of
batch b.  Interleaving the query rows by parity makes the causal structure
identical on every core, so one SPMD program serves all 8 cores; the
parity enters only through the data (a 1-column roll of x^T, cos/sin
tables, and the output row scatter).

Layout strategy (all matmuls contract over the partition dim):
  - x^T   [h, s]   : host-transposed, bf16
  - qkv^T [f, s]   : produced directly by the projection (W rows = contraction)
  - RoPE applied in transposed layout; the even/odd pair interleave is
    converted to a halves layout by permuting W_q / W_k columns on host.
  - scores^T [k, q]: k-tile stationary, q moving -> softmax runs along
    partitions via a DVE accumulation tree + ones-matmul (no transposes).
  - y^T  [d, q]    : v natural-layout stationary, probs^T moving.
  - proj           : y^T slices stationary, W_proj natural moving; output
    lands in [s, o] layout where RMSNorm is a free-dim reduction.
"""

import numpy as np
import ml_dtypes

BF16 = ml_dtypes.bfloat16

# ---------------------------------------------------------------- config
P = 128          # partitions
HD = 128         # head dim
HH = HD // 2     # rope half
G = 4            # GQA group size

B = 4
S = 2048
H = 2048
N_CORES = 8

NH = H // HD          # 16 q heads
NKV = NH // G         # 4 kv heads
KVC = NKV * HD        # 512 kv columns
HT = H // P           # 16 h-tiles (contraction tiles)
S_LOC = S // 2        # 1024 local q rows per core
IT = 512              # i-tile (queries per score tile, = 1 psum bank fp32)
NT_I = S_LOC // IT    # 2 i-slots
SPAN = S // NT_I      # 1024 global rows per slot
JB = SPAN // P        # 8 j-tiles in the diagonal band of each slot
OT = 512              # output-proj column tile
NO = H // OT          # 4

RMS_EPS = 1e-6
SCALE = 1.0 / float(np.sqrt(np.float32(HD)))

_CACHE = {}


# ---------------------------------------------------------------- device IR
def _build_nc():
    from contextlib import ExitStack

    import concourse.bacc as bacc
    import concourse.mybir as mybir
    import concourse.tile as tile

    dt = mybir.dt
    AF = mybir.ActivationFunctionType

    nc = bacc.Bacc("TRN2", target_bir_lowering=False, debug=False)

    xt_d = nc.dram_tensor("xt", [HT, P, S], dt.bfloat16, kind="ExternalInput")
    wq_d = nc.dram_tensor("wq", [NH, P, HT, HD], dt.bfloat16, kind="ExternalInput")
    wk_d = nc.dram_tensor("wk", [NKV, P, HT, HD], dt.bfloat16, kind="ExternalInput")
    wv_d = nc.dram_tensor("wv", [HT, P, KVC], dt.bfloat16, kind="ExternalInput")
    wp_d = nc.dram_tensor("wp", [HT, P, H], dt.bfloat16, kind="ExternalInput")
    qcos_d = nc.dram_tensor("qcos", [HH, S_LOC], dt.float32, kind="ExternalInput")
    qsin_d = nc.dram_tensor("qsin", [HH, S_LOC], dt.float32, kind="ExternalInput")
    kcos_d = nc.dram_tensor("kcos", [HH, S], dt.float32, kind="ExternalInput")
    ksin_d = nc.dram_tensor("ksin", [HH, S], dt.float32, kind="ExternalInput")
    mask_d = nc.dram_tensor("mask", [JB, P, IT], dt.bfloat16, kind="ExternalInput")
    nw_d = nc.dram_tensor("nw", [P, H], dt.float32, kind="ExternalInput")
    out_d = nc.dram_tensor("out", [S_LOC, H], dt.float32, kind="ExternalOutput")

    with tile.TileContext(nc) as tc, ExitStack() as body:
        const = body.enter_context(tc.tile_pool(name="const", bufs=1))
        qcos = const.tile([HH, S_LOC], dt.float32)
        qsin = const.tile([HH, S_LOC], dt.float32)
        kcos = const.tile([HH, S], dt.float32)
        ksin = const.tile([HH, S], dt.float32)
        onesm = const.tile([P, P], dt.bfloat16)
        nc.vector.memset(onesm[:], 1.0)
        epsb = const.tile([P, 1], dt.float32)
        nc.vector.memset(epsb[:], RMS_EPS)
        nc.sync.dma_start(qcos[:], qcos_d.ap())
        nc.sync.dma_start(qsin[:], qsin_d.ap())
        nc.sync.dma_start(kcos[:], kcos_d.ap())
        nc.sync.dma_start(ksin[:], ksin_d.ap())

        s_act = body.enter_context(ExitStack())
        act = s_act.enter_context(tc.tile_pool(name="act", bufs=1))
        qT = act.tile([P, NH * S_LOC], dt.bfloat16)
        kT = act.tile([P, NKV * S], dt.bfloat16)
        vv = act.tile([P, (S // P) * KVC], dt.bfloat16)

        def rope_evict(rpool, ps, dst_lo, dst_hi, cs, sn):
            # dst_lo = ps_lo*cos - ps_hi*sin ; dst_hi = ps_hi*cos + ps_lo*sin
            t1 = rpool.tile([HH, IT], dt.float32, name="rt1")
            t2 = rpool.tile([HH, IT], dt.float32, name="rt2")
            nc.vector.tensor_mul(t1[:], ps[0:HH, :], cs)
            nc.vector.tensor_mul(t2[:], ps[HH:P, :], sn)
            nc.vector.tensor_sub(dst_lo, t1[:], t2[:])
            nc.vector.tensor_mul(t1[:], ps[HH:P, :], cs)
            nc.vector.tensor_mul(t2[:], ps[0:HH, :], sn)
            nc.vector.tensor_add(dst_hi, t1[:], t2[:])

        # ---------------- phase 1: qkv projection + rope --------------
        with ExitStack() as ph1:
            xp = ph1.enter_context(tc.tile_pool(name="xp", bufs=1))
            xt = xp.tile([P, HT * S], dt.bfloat16)
            nc.sync.dma_start(
                xt[:].rearrange("p (t s) -> p t s", t=HT),
                xt_d.ap().rearrange("t p s -> p t s"),
            )

            wkp = ph1.enter_context(tc.tile_pool(name="wkp", bufs=1))
            wk = wkp.tile([P, NKV * HT * HD], dt.bfloat16)
            nc.sync.dma_start(
                wk[:].rearrange("p (f t m) -> p f t m", f=NKV, t=HT),
                wk_d.ap().rearrange("f p t m -> p f t m"),
            )

            wvp = ph1.enter_context(tc.tile_pool(name="wvp", bufs=1))
            wv = wvp.tile([P, HT * KVC], dt.bfloat16)
            nc.sync.dma_start(
                wv[:].rearrange("p (t f) -> p t f", t=HT),
                wv_d.ap().rearrange("t p f -> p t f"),
            )

            wqp = ph1.enter_context(tc.tile_pool(name="wqp", bufs=2))
            rp1 = ph1.enter_context(tc.tile_pool(name="rp1", bufs=2))
            psq = ph1.enter_context(tc.tile_pool(name="psq", bufs=4, space="PSUM"))
            psk = ph1.enter_context(tc.tile_pool(name="psk", bufs=2, space="PSUM"))
            psv = ph1.enter_context(tc.tile_pool(name="psv", bufs=2, space="PSUM"))

            # k projection (all S columns) + rope
            for fk in range(NKV):
                for sc in range(S // IT):
                    ps = psk.tile([P, IT], dt.float32, name="kps")
                    for h in range(HT):
                        nc.tensor.matmul(
                            ps[:],
                            wk[:, fk * H + h * HD : fk * H + (h + 1) * HD],
                            xt[:, h * S + sc * IT : h * S + (sc + 1) * IT],
                            start=(h == 0),
                            stop=(h == HT - 1),
                        )
                    c0 = fk * S + sc * IT
                    rope_evict(
                        rp1, ps,
                        kT[0:HH, c0 : c0 + IT], kT[HH:P, c0 : c0 + IT],
                        kcos[:, sc * IT : (sc + 1) * IT],
                        ksin[:, sc * IT : (sc + 1) * IT],
                    )

            # q projection (local rows, stride-2 reads of x^T) + rope
            for fq in range(NH):
                wq = wqp.tile([P, HT * HD], dt.bfloat16, name="wqt")
                nc.sync.dma_start(
                    wq[:].rearrange("p (t m) -> p t m", t=HT),
                    wq_d.ap()[fq],
                )
                pss = [psq.tile([P, IT], dt.float32, name="qps") for _ in range(NT_I)]
                for h in range(HT):
                    for t in range(NT_I):
                        st = h * S + 2 * t * IT
                        nc.tensor.matmul(
                            pss[t][:],
                            wq[:, h * HD : (h + 1) * HD],
                            xt[:, st : st + 2 * IT : 2],
                            start=(h == 0),
                            stop=(h == HT - 1),
                        )
                for t in range(NT_I):
                    c0 = fq * S_LOC + t * IT
                    rope_evict(
                        rp1, pss[t],
                        qT[0:HH, c0 : c0 + IT], qT[HH:P, c0 : c0 + IT],
                        qcos[:, t * IT : (t + 1) * IT],
                        qsin[:, t * IT : (t + 1) * IT],
                    )

            # v projection (natural [s, f] layout)
            for sv in range(S // P):
                ps = psv.tile([P, KVC], dt.float32, name="vps")
                for h in range(HT):
                    nc.tensor.matmul(
                        ps[:],
                        xt[:, h * S + sv * P : h * S + (sv + 1) * P],
                        wv[:, h * KVC : (h + 1) * KVC],
                        start=(h == 0),
                        stop=(h == HT - 1),
                    )
                nc.scalar.activation(
                    vv[:, sv * KVC : (sv + 1) * KVC], ps[:], AF.Copy
                )

        # ---------------- phase 2: attention --------------------------
        late = body.enter_context(tc.tile_pool(name="late", bufs=1, side="right"))
        masks = late.tile([P, JB * IT], dt.bfloat16)
        nc.sync.dma_start(
            masks[:].rearrange("p (j f) -> p j f", j=JB),
            mask_d.ap().rearrange("j p f -> p j f"),
        )
        nw = late.tile([P, H], dt.float32)
        nc.sync.dma_start(nw[:], nw_d.ap())
        yT = late.tile([P, NH * S_LOC], dt.bfloat16)

        with ExitStack() as ph2:
            prp = ph2.enter_context(tc.tile_pool(name="prp", bufs=18))
            recp = ph2.enter_context(tc.tile_pool(name="recp", bufs=2))
            pss_p = ph2.enter_context(tc.tile_pool(name="pssp", bufs=4, space="PSUM"))
            psy = ph2.enter_context(tc.tile_pool(name="psy", bufs=2, space="PSUM"))
            psd = ph2.enter_context(tc.tile_pool(name="psd", bufs=2, space="PSUM"))

            for hq in range(NH):
                kvh = hq // G
                for t in range(NT_I):
                    nj = (t + 1) * JB
                    yps = psy.tile([P, IT], dt.float32, name="yps")
                    prs = []
                    for j in range(nj):
                        sps = pss_p.tile([P, IT], dt.float32, name="sps")
                        nc.tensor.matmul(
                            sps[:],
                            kT[:, kvh * S + j * P : kvh * S + (j + 1) * P],
                            qT[:, hq * S_LOC + t * IT : hq * S_LOC + (t + 1) * IT],
                            start=True,
                            stop=True,
                        )
                        pr = prp.tile([P, IT], dt.bfloat16, name="pr")
                        nc.scalar.activation(pr[:], sps[:], AF.Exp, scale=SCALE)
                        jj = j - t * JB
                        if jj >= 0:
                            nc.vector.tensor_mul(
                                pr[:], pr[:], masks[:, jj * IT : (jj + 1) * IT]
                            )
                        prs.append(pr)
                        nc.tensor.matmul(
                            yps[:],
                            vv[:, j * KVC + kvh * HD : j * KVC + (kvh + 1) * HD],
                            pr[:],
                            start=(j == 0),
                            stop=(j == nj - 1),
                        )
                    # denominator: all-ones stationary -> column sums,
                    # broadcast across partitions for free
                    dps = psd.tile([P, IT], dt.float32, name="dps")
                    for j in range(nj):
                        nc.tensor.matmul(
                            dps[:], onesm[:], prs[j][:],
                            start=(j == 0), stop=(j == nj - 1),
                        )
                    rec = recp.tile([P, IT], dt.float32, name="rec")
                    nc.vector.reciprocal(rec[:], dps[:])
                    nc.vector.tensor_mul(
                        yT[:, hq * S_LOC + t * IT : hq * S_LOC + (t + 1) * IT],
                        yps[:],
                        rec[:],
                    )

        s_act.close()  # free qT / kT / vv before the projection phase

        # ---------------- phase 3: out projection + rmsnorm ------------
        with ExitStack() as ph3:
            wpp = ph3.enter_context(tc.tile_pool(name="wpp", bufs=1))
            wp = wpp.tile([P, HT * H], dt.bfloat16)
            nc.sync.dma_start(
                wp[:].rearrange("p (t f) -> p t f", t=HT),
                wp_d.ap().rearrange("t p f -> p t f"),
            )
            outp = ph3.enter_context(tc.tile_pool(name="outp", bufs=2))
            sqp = ph3.enter_context(tc.tile_pool(name="sqp", bufs=2))
            smp = ph3.enter_context(tc.tile_pool(name="smp", bufs=2))
            po = ph3.enter_context(tc.tile_pool(name="po", bufs=8, space="PSUM"))

            for sl in range(S_LOC // P):
                pso = [po.tile([P, OT], dt.float32, name="pso") for _ in range(NO)]
                for h in range(HT):
                    lhs = yT[:, h * S_LOC + sl * P : h * S_LOC + (sl + 1) * P]
                    for o in range(NO):
                        nc.tensor.matmul(
                            pso[o][:],
                            lhs,
                            wp[:, h * H + o * OT : h * H + (o + 1) * OT],
                            start=(h == 0),
                            stop=(h == HT - 1),
                        )
                ot = outp.tile([P, H], dt.float32, name="ot")
                for o in range(NO):
                    nc.scalar.activation(
                        ot[:, o * OT : (o + 1) * OT], pso[o][:], AF.Copy
                    )
                sq = sqp.tile([P, H], dt.float32, name="sq")
                ssq = smp.tile([P, 1], dt.float32, name="ssq")
                nc.scalar.activation(sq[:], ot[:], AF.Square, accum_out=ssq[:])
                rms = smp.tile([P, 1], dt.float32, name="rms")
                nc.scalar.activation(
                    rms[:], ssq[:], AF.Sqrt, bias=epsb[:], scale=1.0 / H
                )
                rr = smp.tile([P, 1], dt.float32, name="rr")
                nc.vector.reciprocal(rr[:], rms[:])
                nc.vector.tensor_scalar_mul(ot[:], ot[:], rr[:])
                nc.vector.tensor_mul(ot[:], ot[:], nw[:])
                nc.sync.dma_start(out_d.ap()[sl * P : (sl + 1) * P, :], ot[:])

    nc.compile()
    return nc


# ---------------------------------------------------------------- host side
def _host_shared(w_attn, w_proj, norm_w):
    """Core-independent packed tensors."""
    f32 = np.float32

    def perm_halves(w):  # [H, n, HD] even/odd pairs -> halves
        return np.concatenate([w[..., 0::2], w[..., 1::2]], axis=-1)

    wq = perm_halves(w_attn[:, :H].reshape(H, NH, HD))
    wq = np.ascontiguousarray(
        wq.reshape(HT, P, NH, HD).transpose(2, 1, 0, 3)
    ).astype(BF16)
    wk = perm_halves(w_attn[:, H : H + KVC].reshape(H, NKV, HD))
    wk = np.ascontiguousarray(
        wk.reshape(HT, P, NKV, HD).transpose(2, 1, 0, 3)
    ).astype(BF16)
    wv = np.ascontiguousarray(
        w_attn[:, H + KVC :].reshape(HT, P, KVC)
    ).astype(BF16)
    wp = np.ascontiguousarray(w_proj.reshape(HT, P, H)).astype(BF16)

    jj, p, f = np.meshgrid(
        np.arange(JB), np.arange(P), np.arange(IT), indexing="ij"
    )
    # parity 0: query 2f vs key (128jj + p)
    mask0 = (2 * f >= 128 * jj + p).astype(BF16)
    # parity 1: query 2f+1 vs key (128jj + (p^1))  (pair-swapped x columns)
    mask1 = (2 * f + 1 >= 128 * jj + (p ^ 1)).astype(BF16)

    nw = np.ascontiguousarray(
        np.broadcast_to(norm_w.astype(f32), (P, H))
    )
    return wq, wk, wv, wp, (mask0, mask1), nw


def _cos_sin(pos):
    f32 = np.float32
    inv = 1.0 / (
        10000.0 ** (np.arange(0, HD, 2, dtype=f32) / f32(HD))
    )
    ang = inv[:, None].astype(f32) * pos[None, :].astype(f32)  # [HH, N]
    return np.cos(ang).astype(f32), np.sin(ang).astype(f32)


def make_in_maps(x, w_attn, w_proj, norm_w):
    x = np.asarray(x, dtype=np.float32)
    w_attn = np.asarray(w_attn, dtype=np.float32)
    w_proj = np.asarray(w_proj, dtype=np.float32)
    norm_w = np.asarray(norm_w, dtype=np.float32)

    wq, wk, wv, wp, (mask0, mask1), nw = _host_shared(w_attn, w_proj, norm_w)

    kc0, ks0 = _cos_sin(np.arange(S, dtype=np.float32))          # parity 0
    # parity 1: column j holds global row j^1 (pair-swapped x columns)
    kc1, ks1 = _cos_sin((np.arange(S) ^ 1).astype(np.float32))
    qc0, qs0 = _cos_sin(2.0 * np.arange(S_LOC, dtype=np.float32))
    qc1, qs1 = _cos_sin(2.0 * np.arange(S_LOC, dtype=np.float32) + 1.0)

    in_maps = []
    for c in range(N_CORES):
        b, par = c // 2, c % 2
        xt = x[b].T.astype(BF16)
        if par:
            xt = xt[:, np.arange(S) ^ 1]  # swap adjacent column pairs
        xt = np.ascontiguousarray(xt.reshape(HT, P, S))
        in_maps.append(
            {
                "xt": xt,
                "wq": wq,
                "wk": wk,
                "wv": wv,
                "wp": wp,
                "qcos": qc1 if par else qc0,
                "qsin": qs1 if par else qs0,
                "kcos": kc1 if par else kc0,
                "ksin": ks1 if par else ks0,
                "mask": mask1 if par else mask0,
                "nw": nw,
            }
        )
    return in_maps


def assemble_out(results):
    out = np.empty((B, S, H), dtype=np.float32)
    for c in range(N_CORES):
        b, par = c // 2, c % 2
        out[b, par::2, :] = results[c]["out"]
    return out


def kernel(x, w_attn, w_proj, norm_w):
    from concourse import bass_utils

    if "nc" not in _CACHE:
        _CACHE["nc"] = _build_nc()
    nc = _CACHE["nc"]

    in_maps = make_in_maps(x, w_attn, w_proj, norm_w)
    res = bass_utils.run_bass_kernel_spmd(
        nc, in_maps, core_ids=list(range(N_CORES))
    )
    return assemble_out(res.results)


# revision 43
# speedup vs baseline: 1.8019x; 1.0279x over previous
"""Trainium2 Bass kernel for nn_Attention_8366596292664.

Dense transformer block: qkv proj -> RoPE -> GQA causal attention ->
out proj -> RMSNorm.  B=4, S=2048, H=2048, 16 heads (hd=128), 4 KV heads.

Sharding: 8 cores = (4 batches) x (2 interleaved query-row parities).
Core (b, par) computes the full block for query rows {par, par+2, ...} of
batch b.  Interleaving the query rows by parity makes the causal structure
identical on every core, so one SPMD program serves all 8 cores; the
parity enters only through the data (a 1-column roll of x^T, cos/sin
tables, and the output row scatter).

Layout strategy (all matmuls contract over the partition dim):
  - x^T   [h, s]   : host-transposed, bf16
  - qkv^T [f, s]   : produced directly by the projection (W rows = contraction)
  - RoPE applied in transposed layout; the even/odd pair interleave is
    converted to a halves layout by permuting W_q / W_k columns on host.
  - scores^T [k, q]: k-tile stationary, q moving -> softmax runs along
    partitions via a DVE accumulation tree + ones-matmul (no transposes).
  - y^T  [d, q]    : v natural-layout stationary, probs^T moving.
  - proj           : y^T slices stationary, W_proj natural moving; output
    lands in [s, o] layout where RMSNorm is a free-dim reduction.
"""

import numpy as np
import ml_dtypes

BF16 = ml_dtypes.bfloat16

# ---------------------------------------------------------------- config
P = 128          # partitions
HD = 128         # head dim
HH = HD // 2     # rope half
G = 4            # GQA group size

B = 4
S = 2048
H = 2048
N_CORES = 8

NH = H // HD          # 16 q heads
NKV = NH // G         # 4 kv heads
KVC = NKV * HD        # 512 kv columns
HT = H // P           # 16 h-tiles (contraction tiles)
S_LOC = S // 2        # 1024 local q rows per core
IT = 512              # i-tile (queries per score tile, = 1 psum bank fp32)
NT_I = S_LOC // IT    # 2 i-slots
SPAN = S // NT_I      # 1024 global rows per slot
JB = SPAN // P        # 8 j-tiles in the diagonal band of each slot
OT = 512              # output-proj column tile
NO = H // OT          # 4

RMS_EPS = 1e-6
SCALE = 1.0 / float(np.sqrt(np.float32(HD)))

_CACHE = {}


# ---------------------------------------------------------------- device IR
def _build_nc():
    from contextlib import ExitStack

    import concourse.bacc as bacc
    import concourse.mybir as mybir
    import concourse.tile as tile

    dt = mybir.dt
    AF = mybir.ActivationFunctionType

    nc = bacc.Bacc("TRN2", target_bir_lowering=False, debug=False)

    xt_d = nc.dram_tensor("xt", [HT, P, S], dt.bfloat16, kind="ExternalInput")
    wq_d = nc.dram_tensor("wq", [NH, P, HT, HD], dt.bfloat16, kind="ExternalInput")
    wk_d = nc.dram_tensor("wk", [NKV, P, HT, HD], dt.bfloat16, kind="ExternalInput")
    wv_d = nc.dram_tensor("wv", [HT, P, KVC], dt.bfloat16, kind="ExternalInput")
    wp_d = nc.dram_tensor("wp", [HT, P, H], dt.bfloat16, kind="ExternalInput")
    qcos_d = nc.dram_tensor("qcos", [P, S_LOC], dt.bfloat16, kind="ExternalInput")
    qsin_d = nc.dram_tensor("qsin", [P, S_LOC], dt.bfloat16, kind="ExternalInput")
    kcos_d = nc.dram_tensor("kcos", [P, S], dt.bfloat16, kind="ExternalInput")
    ksin_d = nc.dram_tensor("ksin", [P, S], dt.bfloat16, kind="ExternalInput")
    mask_d = nc.dram_tensor("mask", [JB, P, IT], dt.bfloat16, kind="ExternalInput")
    nw_d = nc.dram_tensor("nw", [P, H], dt.float32, kind="ExternalInput")
    out_d = nc.dram_tensor("out", [S_LOC, H], dt.float32, kind="ExternalOutput")

    with tile.TileContext(nc) as tc, ExitStack() as body:
        const = body.enter_context(tc.tile_pool(name="const", bufs=1))
        qcos = const.tile([P, S_LOC], dt.bfloat16)
        qsin = const.tile([P, S_LOC], dt.bfloat16)
        kcos = const.tile([P, S], dt.bfloat16)
        ksin = const.tile([P, S], dt.bfloat16)
        onesm = const.tile([P, P], dt.bfloat16)
        nc.vector.memset(onesm[:], 1.0)
        epsb = const.tile([P, 1], dt.float32)
        nc.vector.memset(epsb[:], RMS_EPS)
        nc.sync.dma_start(qcos[:], qcos_d.ap())
        nc.sync.dma_start(qsin[:], qsin_d.ap())
        nc.sync.dma_start(kcos[:], kcos_d.ap())
        nc.sync.dma_start(ksin[:], ksin_d.ap())

        s_act = body.enter_context(ExitStack())
        act = s_act.enter_context(tc.tile_pool(name="act", bufs=1))
        qT = act.tile([P, NH * S_LOC], dt.bfloat16)
        kT = act.tile([P, NKV * S], dt.bfloat16)
        vv = act.tile([P, (S // P) * KVC], dt.bfloat16)

        def rope_evict(rpool, ps, dst_lo, dst_hi, cs, sn):
            # dst_lo = ps_lo*cos - ps_hi*sin ; dst_hi = ps_hi*cos + ps_lo*sin
            # Stage PSUM -> SBUF bf16 on ACT so the DVE ops run in the
            # all-bf16 SBUF fast path.
            stg = rpool.tile([P, IT], dt.bfloat16, name="rstg")
            nc.scalar.activation(stg[:], ps[:], AF.Copy)
            t1 = rpool.tile([HH, IT], dt.bfloat16, name="rt1")
            t2 = rpool.tile([HH, IT], dt.bfloat16, name="rt2")
            nc.vector.tensor_mul(t1[:], stg[0:HH, :], cs[0:HH, :])
            nc.vector.tensor_mul(t2[:], stg[HH:P, :], sn[HH:P, :])
            nc.vector.tensor_sub(dst_lo, t1[:], t2[:])
            nc.vector.tensor_mul(t1[:], stg[HH:P, :], cs[HH:P, :])
            nc.vector.tensor_mul(t2[:], stg[0:HH, :], sn[0:HH, :])
            nc.vector.tensor_add(dst_hi, t1[:], t2[:])

        # ---------------- phase 1: qkv projection + rope --------------
        with ExitStack() as ph1:
            xp = ph1.enter_context(tc.tile_pool(name="xp", bufs=1))
            xt = xp.tile([P, HT * S], dt.bfloat16)
            nc.sync.dma_start(
                xt[:].rearrange("p (t s) -> p t s", t=HT),
                xt_d.ap().rearrange("t p s -> p t s"),
            )

            wkp = ph1.enter_context(tc.tile_pool(name="wkp", bufs=1))
            wk = wkp.tile([P, NKV * HT * HD], dt.bfloat16)
            nc.sync.dma_start(
                wk[:].rearrange("p (f t m) -> p f t m", f=NKV, t=HT),
                wk_d.ap().rearrange("f p t m -> p f t m"),
            )

            wvp = ph1.enter_context(tc.tile_pool(name="wvp", bufs=1))
            wv = wvp.tile([P, HT * KVC], dt.bfloat16)
            nc.sync.dma_start(
                wv[:].rearrange("p (t f) -> p t f", t=HT),
                wv_d.ap().rearrange("t p f -> p t f"),
            )

            wqp = ph1.enter_context(tc.tile_pool(name="wqp", bufs=2))
            rp1 = ph1.enter_context(tc.tile_pool(name="rp1", bufs=2))
            psq = ph1.enter_context(tc.tile_pool(name="psq", bufs=4, space="PSUM"))
            psk = ph1.enter_context(tc.tile_pool(name="psk", bufs=2, space="PSUM"))
            psv = ph1.enter_context(tc.tile_pool(name="psv", bufs=2, space="PSUM"))

            # k projection (all S columns) + rope
            for fk in range(NKV):
                for sc in range(S // IT):
                    ps = psk.tile([P, IT], dt.float32, name="kps")
                    for h in range(HT):
                        nc.tensor.matmul(
                            ps[:],
                            wk[:, fk * H + h * HD : fk * H + (h + 1) * HD],
                            xt[:, h * S + sc * IT : h * S + (sc + 1) * IT],
                            start=(h == 0),
                            stop=(h == HT - 1),
                        )
                    c0 = fk * S + sc * IT
                    rope_evict(
                        rp1, ps,
                        kT[0:HH, c0 : c0 + IT], kT[HH:P, c0 : c0 + IT],
                        kcos[:, sc * IT : (sc + 1) * IT],
                        ksin[:, sc * IT : (sc + 1) * IT],
                    )

            # q projection (local rows, stride-2 reads of x^T) + rope
            for fq in range(NH):
                wq = wqp.tile([P, HT * HD], dt.bfloat16, name="wqt")
                nc.sync.dma_start(
                    wq[:].rearrange("p (t m) -> p t m", t=HT),
                    wq_d.ap()[fq],
                )
                pss = [psq.tile([P, IT], dt.float32, name="qps") for _ in range(NT_I)]
                for h in range(HT):
                    for t in range(NT_I):
                        st = h * S + 2 * t * IT
                        nc.tensor.matmul(
                            pss[t][:],
                            wq[:, h * HD : (h + 1) * HD],
                            xt[:, st : st + 2 * IT : 2],
                            start=(h == 0),
                            stop=(h == HT - 1),
                        )
                for t in range(NT_I):
                    c0 = fq * S_LOC + t * IT
                    rope_evict(
                        rp1, pss[t],
                        qT[0:HH, c0 : c0 + IT], qT[HH:P, c0 : c0 + IT],
                        qcos[:, t * IT : (t + 1) * IT],
                        qsin[:, t * IT : (t + 1) * IT],
                    )

            # v projection (natural [s, f] layout)
            for sv in range(S // P):
                ps = psv.tile([P, KVC], dt.float32, name="vps")
                for h in range(HT):
                    nc.tensor.matmul(
                        ps[:],
                        xt[:, h * S + sv * P : h * S + (sv + 1) * P],
                        wv[:, h * KVC : (h + 1) * KVC],
                        start=(h == 0),
                        stop=(h == HT - 1),
                    )
                nc.scalar.activation(
                    vv[:, sv * KVC : (sv + 1) * KVC], ps[:], AF.Copy
                )

        # ---------------- phase 2: attention --------------------------
        late = body.enter_context(tc.tile_pool(name="late", bufs=1, side="right"))
        masks = late.tile([P, JB * IT], dt.bfloat16)
        nc.sync.dma_start(
            masks[:].rearrange("p (j f) -> p j f", j=JB),
            mask_d.ap().rearrange("j p f -> p j f"),
        )
        nw = late.tile([P, H], dt.float32)
        nc.sync.dma_start(nw[:], nw_d.ap())
        yT = late.tile([P, NH * S_LOC], dt.bfloat16)

        with ExitStack() as ph2:
            prp = ph2.enter_context(tc.tile_pool(name="prp", bufs=9))
            recp = ph2.enter_context(tc.tile_pool(name="recp", bufs=2))
            pss_p = ph2.enter_context(tc.tile_pool(name="pssp", bufs=2, space="PSUM"))
            psy = ph2.enter_context(tc.tile_pool(name="psy", bufs=2, space="PSUM"))
            psd = ph2.enter_context(tc.tile_pool(name="psd", bufs=2, space="PSUM"))

            for hq in range(NH):
                kvh = hq // G
                for t in range(NT_I):
                    nj = (t + 1) * JB
                    qslice = qT[:, hq * S_LOC + t * IT : hq * S_LOC + (t + 1) * IT]
                    yps = psy.tile([P, IT], dt.float32, name="yps")
                    prs = []
                    for jp in range(nj // 2):
                        j0 = 2 * jp
                        sps = pss_p.tile([P, 2 * IT], dt.float32, name="sps")
                        for u in range(2):
                            nc.tensor.matmul(
                                sps[:, u * IT : (u + 1) * IT],
                                kT[:, kvh * S + (j0 + u) * P : kvh * S + (j0 + u + 1) * P],
                                qslice,
                                start=True,
                                stop=True,
                            )
                        pr = prp.tile([P, 2 * IT], dt.bfloat16, name="pr")
                        nc.scalar.activation(pr[:], sps[:], AF.Exp, scale=SCALE)
                        jj = j0 - t * JB
                        if jj >= 0:
                            nc.vector.tensor_mul(
                                pr[:], pr[:], masks[:, jj * IT : (jj + 2) * IT]
                            )
                        prs.append(pr)
                        for u in range(2):
                            j = j0 + u
                            nc.tensor.matmul(
                                yps[:],
                                vv[:, j * KVC + kvh * HD : j * KVC + (kvh + 1) * HD],
                                pr[:, u * IT : (u + 1) * IT],
                                start=(j == 0),
                                stop=(j == nj - 1),
                            )
                    # denominator: all-ones stationary -> column sums,
                    # broadcast across partitions for free
                    dps = psd.tile([P, IT], dt.float32, name="dps")
                    for jp in range(nj // 2):
                        for u in range(2):
                            j = 2 * jp + u
                            nc.tensor.matmul(
                                dps[:], onesm[:],
                                prs[jp][:, u * IT : (u + 1) * IT],
                                start=(j == 0), stop=(j == nj - 1),
                            )
                    rec = recp.tile([P, IT], dt.float32, name="rec")
                    nc.vector.reciprocal_approx_fast(rec[:], dps[:])
                    nc.vector.tensor_mul(
                        yT[:, hq * S_LOC + t * IT : hq * S_LOC + (t + 1) * IT],
                        yps[:],
                        rec[:],
                    )

        s_act.close()  # free qT / kT / vv before the projection phase

        # ---------------- phase 3: out projection + rmsnorm ------------
        with ExitStack() as ph3:
            wpp = ph3.enter_context(tc.tile_pool(name="wpp", bufs=1))
            wp = wpp.tile([P, HT * H], dt.bfloat16)
            nc.sync.dma_start(
                wp[:].rearrange("p (t f) -> p t f", t=HT),
                wp_d.ap().rearrange("t p f -> p t f"),
            )
            outp = ph3.enter_context(tc.tile_pool(name="outp", bufs=2))
            sqp = ph3.enter_context(tc.tile_pool(name="sqp", bufs=2))
            smp = ph3.enter_context(tc.tile_pool(name="smp", bufs=2))
            po = ph3.enter_context(tc.tile_pool(name="po", bufs=8, space="PSUM"))

            for sl in range(S_LOC // P):
                pso = [po.tile([P, OT], dt.float32, name="pso") for _ in range(NO)]
                for h in range(HT):
                    lhs = yT[:, h * S_LOC + sl * P : h * S_LOC + (sl + 1) * P]
                    for o in range(NO):
                        nc.tensor.matmul(
                            pso[o][:],
                            lhs,
                            wp[:, h * H + o * OT : h * H + (o + 1) * OT],
                            start=(h == 0),
                            stop=(h == HT - 1),
                        )
                ot = outp.tile([P, H], dt.float32, name="ot")
                for o in range(NO):
                    nc.scalar.activation(
                        ot[:, o * OT : (o + 1) * OT], pso[o][:], AF.Copy
                    )
                sq = sqp.tile([P, H], dt.float32, name="sq")
                ssq = smp.tile([P, 1], dt.float32, name="ssq")
                nc.scalar.activation(sq[:], ot[:], AF.Square, accum_out=ssq[:])
                rms = smp.tile([P, 1], dt.float32, name="rms")
                nc.scalar.activation(
                    rms[:], ssq[:], AF.Sqrt, bias=epsb[:], scale=1.0 / H
                )
                rr = smp.tile([P, 1], dt.float32, name="rr")
                nc.vector.reciprocal(rr[:], rms[:])
                nc.vector.tensor_scalar_mul(ot[:], ot[:], rr[:])
                nc.vector.tensor_mul(ot[:], ot[:], nw[:])
                nc.sync.dma_start(out_d.ap()[sl * P : (sl + 1) * P, :], ot[:])

    nc.compile()
    return nc


# ---------------------------------------------------------------- host side
def _host_shared(w_attn, w_proj, norm_w):
    """Core-independent packed tensors."""
    f32 = np.float32

    def perm_halves(w):  # [H, n, HD] even/odd pairs -> halves
        return np.concatenate([w[..., 0::2], w[..., 1::2]], axis=-1)

    wq = perm_halves(w_attn[:, :H].reshape(H, NH, HD))
    wq = np.ascontiguousarray(
        wq.reshape(HT, P, NH, HD).transpose(2, 1, 0, 3)
    ).astype(BF16)
    wk = perm_halves(w_attn[:, H : H + KVC].reshape(H, NKV, HD))
    wk = np.ascontiguousarray(
        wk.reshape(HT, P, NKV, HD).transpose(2, 1, 0, 3)
    ).astype(BF16)
    wv = np.ascontiguousarray(
        w_attn[:, H + KVC :].reshape(HT, P, KVC)
    ).astype(BF16)
    wp = np.ascontiguousarray(w_proj.reshape(HT, P, H)).astype(BF16)

    jj, p, f = np.meshgrid(
        np.arange(JB), np.arange(P), np.arange(IT), indexing="ij"
    )
    # parity 0: query 2f vs key (128jj + p)
    mask0 = (2 * f >= 128 * jj + p).astype(BF16)
    # parity 1: query 2f+1 vs key (128jj + (p^1))  (pair-swapped x columns)
    mask1 = (2 * f + 1 >= 128 * jj + (p ^ 1)).astype(BF16)

    nw = np.ascontiguousarray(
        np.broadcast_to(norm_w.astype(f32), (P, H))
    )
    return wq, wk, wv, wp, (mask0, mask1), nw


def _cos_sin(pos):
    f32 = np.float32
    inv = 1.0 / (
        10000.0 ** (np.arange(0, HD, 2, dtype=f32) / f32(HD))
    )
    ang = inv[:, None].astype(f32) * pos[None, :].astype(f32)  # [HH, N]
    c, s = np.cos(ang).astype(BF16), np.sin(ang).astype(BF16)
    # duplicated across both partition halves (walrus wants equal base
    # partitions for SBUF tensor-tensor inputs)
    return (
        np.ascontiguousarray(np.concatenate([c, c], axis=0)),
        np.ascontiguousarray(np.concatenate([s, s], axis=0)),
    )


def make_in_maps(x, w_attn, w_proj, norm_w):
    x = np.asarray(x, dtype=np.float32)
    w_attn = np.asarray(w_attn, dtype=np.float32)
    w_proj = np.asarray(w_proj, dtype=np.float32)
    norm_w = np.asarray(norm_w, dtype=np.float32)

    wq, wk, wv, wp, (mask0, mask1), nw = _host_shared(w_attn, w_proj, norm_w)

    kc0, ks0 = _cos_sin(np.arange(S, dtype=np.float32))          # parity 0
    # parity 1: column j holds global row j^1 (pair-swapped x columns)
    kc1, ks1 = _cos_sin((np.arange(S) ^ 1).astype(np.float32))
    qc0, qs0 = _cos_sin(2.0 * np.arange(S_LOC, dtype=np.float32))
    qc1, qs1 = _cos_sin(2.0 * np.arange(S_LOC, dtype=np.float32) + 1.0)

    in_maps = []
    for c in range(N_CORES):
        b, par = c // 2, c % 2
        xt = x[b].T.astype(BF16)
        if par:
            xt = xt[:, np.arange(S) ^ 1]  # swap adjacent column pairs
        xt = np.ascontiguousarray(xt.reshape(HT, P, S))
        in_maps.append(
            {
                "xt": xt,
                "wq": wq,
                "wk": wk,
                "wv": wv,
                "wp": wp,
                "qcos": qc1 if par else qc0,
                "qsin": qs1 if par else qs0,
                "kcos": kc1 if par else kc0,
                "ksin": ks1 if par else ks0,
                "mask": mask1 if par else mask0,
                "nw": nw,
            }
        )
    return in_maps


def assemble_out(results):
    out = np.empty((B, S, H), dtype=np.float32)
    for c in range(N_CORES):
        b, par = c // 2, c % 2
        out[b, par::2, :] = results[c]["out"]
    return out


def kernel(x, w_attn, w_proj, norm_w):
    from concourse import bass_utils

    if "nc" not in _CACHE:
        _CACHE["nc"] = _build_nc()
    nc = _CACHE["nc"]

    in_maps = make_in_maps(x, w_attn, w_proj, norm_w)
    res = bass_utils.run_bass_kernel_spmd(
        nc, in_maps, core_ids=list(range(N_CORES))
    )
    return assemble_out(res.results)


# revision 46
# speedup vs baseline: 1.9580x; 1.0866x over previous
"""Trainium2 Bass kernel for nn_Attention_8366596292664.

Dense transformer block: qkv proj -> RoPE -> GQA causal attention ->
out proj -> RMSNorm.  B=4, S=2048, H=2048, 16 heads (hd=128), 4 KV heads.

Sharding: 8 cores = (4 batches) x (2 interleaved query-row parities).
Core (b, par) computes the full block for query rows {par, par+2, ...} of
batch b.  Interleaving the query rows by parity makes the causal structure
identical on every core, so one SPMD program serves all 8 cores; the
parity enters only through the data (a 1-column roll of x^T, cos/sin
tables, and the output row scatter).

Layout strategy (all matmuls contract over the partition dim):
  - x^T   [h, s]   : host-transposed, bf16
  - qkv^T [f, s]   : produced directly by the projection (W rows = contraction)
  - RoPE applied in transposed layout; the even/odd pair interleave is
    converted to a halves layout by permuting W_q / W_k columns on host.
  - scores^T [k, q]: k-tile stationary, q moving -> softmax runs along
    partitions via a DVE accumulation tree + ones-matmul (no transposes).
  - y^T  [d, q]    : v natural-layout stationary, probs^T moving.
  - proj           : y^T slices stationary, W_proj natural moving; output
    lands in [s, o] layout where RMSNorm is a free-dim reduction.
"""

import numpy as np
import ml_dtypes

BF16 = ml_dtypes.bfloat16

# ---------------------------------------------------------------- config
P = 128          # partitions
HD = 128         # head dim
HH = HD // 2     # rope half
G = 4            # GQA group size

B = 4
S = 2048
H = 2048
N_CORES = 8

NH = H // HD          # 16 q heads
NKV = NH // G         # 4 kv heads
KVC = NKV * HD        # 512 kv columns
HT = H // P           # 16 h-tiles (contraction tiles)
S_LOC = S // 2        # 1024 local q rows per core
IT = 512              # i-tile (queries per score tile, = 1 psum bank fp32)
NT_I = S_LOC // IT    # 2 i-slots
SPAN = S // NT_I      # 1024 global rows per slot
JB = SPAN // P        # 8 j-tiles in the diagonal band of each slot
OT = 512              # output-proj column tile
NO = H // OT          # 4

RMS_EPS = 1e-6
SCALE = 1.0 / float(np.sqrt(np.float32(HD)))

_CACHE = {}


# ---------------------------------------------------------------- device IR
def _build_nc():
    from contextlib import ExitStack

    import concourse.bacc as bacc
    import concourse.mybir as mybir
    import concourse.tile as tile

    dt = mybir.dt
    AF = mybir.ActivationFunctionType

    nc = bacc.Bacc("TRN2", target_bir_lowering=False, debug=False)

    xt_d = nc.dram_tensor("xt", [HT, P, S], dt.bfloat16, kind="ExternalInput")
    wq_d = nc.dram_tensor("wq", [NH, P, HT, HD], dt.bfloat16, kind="ExternalInput")
    wk_d = nc.dram_tensor("wk", [NKV, P, HT, HD], dt.bfloat16, kind="ExternalInput")
    wv_d = nc.dram_tensor("wv", [HT, P, KVC], dt.bfloat16, kind="ExternalInput")
    wp_d = nc.dram_tensor("wp", [HT, P, H], dt.bfloat16, kind="ExternalInput")
    qcos_d = nc.dram_tensor("qcos", [P, S_LOC], dt.bfloat16, kind="ExternalInput")
    qsin_d = nc.dram_tensor("qsin", [P, S_LOC], dt.bfloat16, kind="ExternalInput")
    kcos_d = nc.dram_tensor("kcos", [P, S], dt.bfloat16, kind="ExternalInput")
    ksin_d = nc.dram_tensor("ksin", [P, S], dt.bfloat16, kind="ExternalInput")
    mask_d = nc.dram_tensor("mask", [JB, P, IT], dt.bfloat16, kind="ExternalInput")
    nw_d = nc.dram_tensor("nw", [P, H], dt.float32, kind="ExternalInput")
    out_d = nc.dram_tensor("out", [S_LOC, H], dt.float32, kind="ExternalOutput")

    with tile.TileContext(nc) as tc, ExitStack() as body:
        const = body.enter_context(tc.tile_pool(name="const", bufs=1))
        qcos = const.tile([P, S_LOC], dt.bfloat16)
        qsin = const.tile([P, S_LOC], dt.bfloat16)
        kcos = const.tile([P, S], dt.bfloat16)
        ksin = const.tile([P, S], dt.bfloat16)
        onesm = const.tile([P, P], dt.bfloat16)
        nc.vector.memset(onesm[:], 1.0)
        epsb = const.tile([P, 1], dt.float32)
        nc.vector.memset(epsb[:], RMS_EPS)
        nc.sync.dma_start(qcos[:], qcos_d.ap())
        nc.sync.dma_start(qsin[:], qsin_d.ap())
        nc.sync.dma_start(kcos[:], kcos_d.ap())
        nc.sync.dma_start(ksin[:], ksin_d.ap())

        s_act = body.enter_context(ExitStack())
        act = s_act.enter_context(tc.tile_pool(name="act", bufs=1))
        qT = act.tile([P, NH * S_LOC], dt.bfloat16)
        kT = act.tile([P, NKV * S], dt.bfloat16)
        vv = act.tile([P, (S // P) * KVC], dt.bfloat16)

        def rope_evict(rpool, ps, dst_lo, dst_hi, cs, sn):
            # dst_lo = ps_lo*cos - ps_hi*sin ; dst_hi = ps_hi*cos + ps_lo*sin
            # Stage PSUM -> SBUF bf16 on ACT so the DVE ops run in the
            # all-bf16 SBUF fast path.
            stg = rpool.tile([P, IT], dt.bfloat16, name="rstg")
            nc.scalar.activation(stg[:], ps[:], AF.Copy)
            t1 = rpool.tile([HH, IT], dt.bfloat16, name="rt1")
            t2 = rpool.tile([HH, IT], dt.bfloat16, name="rt2")
            nc.vector.tensor_mul(t1[:], stg[0:HH, :], cs[0:HH, :])
            nc.vector.tensor_mul(t2[:], stg[HH:P, :], sn[HH:P, :])
            nc.vector.tensor_sub(dst_lo, t1[:], t2[:])
            nc.vector.tensor_mul(t1[:], stg[HH:P, :], cs[HH:P, :])
            nc.vector.tensor_mul(t2[:], stg[0:HH, :], sn[0:HH, :])
            nc.vector.tensor_add(dst_hi, t1[:], t2[:])

        # ---------------- phase 1: qkv projection + rope --------------
        with ExitStack() as ph1:
            xp = ph1.enter_context(tc.tile_pool(name="xp", bufs=1))
            xt = xp.tile([P, HT * S], dt.bfloat16)
            nc.sync.dma_start(
                xt[:].rearrange("p (t s) -> p t s", t=HT),
                xt_d.ap().rearrange("t p s -> p t s"),
            )

            wkp = ph1.enter_context(tc.tile_pool(name="wkp", bufs=1))
            wk = wkp.tile([P, NKV * HT * HD], dt.bfloat16)
            nc.sync.dma_start(
                wk[:].rearrange("p (f t m) -> p f t m", f=NKV, t=HT),
                wk_d.ap().rearrange("f p t m -> p f t m"),
            )

            wvp = ph1.enter_context(tc.tile_pool(name="wvp", bufs=1))
            wv = wvp.tile([P, HT * KVC], dt.bfloat16)
            nc.sync.dma_start(
                wv[:].rearrange("p (t f) -> p t f", t=HT),
                wv_d.ap().rearrange("t p f -> p t f"),
            )

            wqp = ph1.enter_context(tc.tile_pool(name="wqp", bufs=2))
            rp1 = ph1.enter_context(tc.tile_pool(name="rp1", bufs=2))
            psq = ph1.enter_context(tc.tile_pool(name="psq", bufs=4, space="PSUM"))
            psk = ph1.enter_context(tc.tile_pool(name="psk", bufs=2, space="PSUM"))
            psv = ph1.enter_context(tc.tile_pool(name="psv", bufs=2, space="PSUM"))

            # k projection (all S columns) + rope
            for fk in range(NKV):
                for sc in range(S // IT):
                    ps = psk.tile([P, IT], dt.float32, name="kps")
                    for h in range(HT):
                        nc.tensor.matmul(
                            ps[:],
                            wk[:, fk * H + h * HD : fk * H + (h + 1) * HD],
                            xt[:, h * S + sc * IT : h * S + (sc + 1) * IT],
                            start=(h == 0),
                            stop=(h == HT - 1),
                        )
                    c0 = fk * S + sc * IT
                    rope_evict(
                        rp1, ps,
                        kT[0:HH, c0 : c0 + IT], kT[HH:P, c0 : c0 + IT],
                        kcos[:, sc * IT : (sc + 1) * IT],
                        ksin[:, sc * IT : (sc + 1) * IT],
                    )

            # v projection (natural [s, f] layout)
            for sv in range(S // P):
                ps = psv.tile([P, KVC], dt.float32, name="vps")
                for h in range(HT):
                    nc.tensor.matmul(
                        ps[:],
                        xt[:, h * S + sv * P : h * S + (sv + 1) * P],
                        wv[:, h * KVC : (h + 1) * KVC],
                        start=(h == 0),
                        stop=(h == HT - 1),
                    )
                nc.scalar.activation(
                    vv[:, sv * KVC : (sv + 1) * KVC], ps[:], AF.Copy
                )

            # q projection (local rows, stride-2 reads of x^T) + rope.
            # Emitted last so attention on head h can start as soon as
            # head h's q is ready.
            for fq in range(NH):
                wq = wqp.tile([P, HT * HD], dt.bfloat16, name="wqt")
                nc.sync.dma_start(
                    wq[:].rearrange("p (t m) -> p t m", t=HT),
                    wq_d.ap()[fq],
                )
                pss = [psq.tile([P, IT], dt.float32, name="qps") for _ in range(NT_I)]
                for h in range(HT):
                    for t in range(NT_I):
                        st = h * S + 2 * t * IT
                        nc.tensor.matmul(
                            pss[t][:],
                            wq[:, h * HD : (h + 1) * HD],
                            xt[:, st : st + 2 * IT : 2],
                            start=(h == 0),
                            stop=(h == HT - 1),
                        )
                for t in range(NT_I):
                    c0 = fq * S_LOC + t * IT
                    rope_evict(
                        rp1, pss[t],
                        qT[0:HH, c0 : c0 + IT], qT[HH:P, c0 : c0 + IT],
                        qcos[:, t * IT : (t + 1) * IT],
                        qsin[:, t * IT : (t + 1) * IT],
                    )

        # ---------------- phase 2: attention --------------------------
        late = body.enter_context(tc.tile_pool(name="late", bufs=1, side="right"))
        masks = late.tile([P, JB * IT], dt.bfloat16)
        nc.sync.dma_start(
            masks[:].rearrange("p (j f) -> p j f", j=JB),
            mask_d.ap().rearrange("j p f -> p j f"),
        )
        nw = late.tile([P, H], dt.float32)
        nc.sync.dma_start(nw[:], nw_d.ap())
        yT = late.tile([P, NH * S_LOC], dt.bfloat16)

        with ExitStack() as ph2:
            prp = ph2.enter_context(tc.tile_pool(name="prp", bufs=9))
            dsp = ph2.enter_context(tc.tile_pool(name="dsp", bufs=9))
            recp = ph2.enter_context(tc.tile_pool(name="recp", bufs=2))
            pss_p = ph2.enter_context(tc.tile_pool(name="pssp", bufs=2, space="PSUM"))
            psy = ph2.enter_context(tc.tile_pool(name="psy", bufs=2, space="PSUM"))
            psd = ph2.enter_context(tc.tile_pool(name="psd", bufs=2, space="PSUM"))

            for hq in range(NH):
                kvh = hq // G
                for t in range(NT_I):
                    nj = (t + 1) * JB
                    qslice = qT[:, hq * S_LOC + t * IT : hq * S_LOC + (t + 1) * IT]
                    yps = psy.tile([P, IT], dt.float32, name="yps")
                    prs = []
                    for jp in range(nj // 2):
                        j0 = 2 * jp
                        sps = pss_p.tile([P, 2 * IT], dt.float32, name="sps")
                        for u in range(2):
                            nc.tensor.matmul(
                                sps[:, u * IT : (u + 1) * IT],
                                kT[:, kvh * S + (j0 + u) * P : kvh * S + (j0 + u + 1) * P],
                                qslice,
                                start=True,
                                stop=True,
                            )
                        pr = prp.tile([P, 2 * IT], dt.bfloat16, name="pr")
                        nc.scalar.activation(pr[:], sps[:], AF.Exp, scale=SCALE)
                        jj = j0 - t * JB
                        if jj >= 0:
                            nc.vector.tensor_mul(
                                pr[:], pr[:], masks[:, jj * IT : (jj + 2) * IT]
                            )
                        # pair-sum on DVE halves the denominator matmuls
                        ds = dsp.tile([P, IT], dt.bfloat16, name="ds")
                        nc.vector.tensor_add(
                            ds[:], pr[:, 0:IT], pr[:, IT : 2 * IT]
                        )
                        prs.append(ds)
                        for u in range(2):
                            j = j0 + u
                            nc.tensor.matmul(
                                yps[:],
                                vv[:, j * KVC + kvh * HD : j * KVC + (kvh + 1) * HD],
                                pr[:, u * IT : (u + 1) * IT],
                                start=(j == 0),
                                stop=(j == nj - 1),
                            )
                    # denominator: all-ones stationary -> column sums,
                    # broadcast across partitions for free
                    dps = psd.tile([P, IT], dt.float32, name="dps")
                    for jp in range(nj // 2):
                        nc.tensor.matmul(
                            dps[:], onesm[:], prs[jp][:],
                            start=(jp == 0), stop=(jp == nj // 2 - 1),
                        )
                    rec = recp.tile([P, IT], dt.float32, name="rec")
                    nc.vector.reciprocal_approx_fast(rec[:], dps[:])
                    nc.vector.tensor_mul(
                        yT[:, hq * S_LOC + t * IT : hq * S_LOC + (t + 1) * IT],
                        yps[:],
                        rec[:],
                    )

        s_act.close()  # free qT / kT / vv before the projection phase

        # ---------------- phase 3: out projection + rmsnorm ------------
        with ExitStack() as ph3:
            wpp = ph3.enter_context(tc.tile_pool(name="wpp", bufs=1))
            wp = wpp.tile([P, HT * H], dt.bfloat16)
            nc.sync.dma_start(
                wp[:].rearrange("p (t f) -> p t f", t=HT),
                wp_d.ap().rearrange("t p f -> p t f"),
            )
            outp = ph3.enter_context(tc.tile_pool(name="outp", bufs=2))
            sqp = ph3.enter_context(tc.tile_pool(name="sqp", bufs=2))
            smp = ph3.enter_context(tc.tile_pool(name="smp", bufs=2))
            po = ph3.enter_context(tc.tile_pool(name="po", bufs=8, space="PSUM"))

            for sl in range(S_LOC // P):
                pso = [po.tile([P, OT], dt.float32, name="pso") for _ in range(NO)]
                for h in range(HT):
                    lhs = yT[:, h * S_LOC + sl * P : h * S_LOC + (sl + 1) * P]
                    for o in range(NO):
                        nc.tensor.matmul(
                            pso[o][:],
                            lhs,
                            wp[:, h * H + o * OT : h * H + (o + 1) * OT],
                            start=(h == 0),
                            stop=(h == HT - 1),
                        )
                ot = outp.tile([P, H], dt.float32, name="ot")
                for o in range(NO):
                    nc.scalar.activation(
                        ot[:, o * OT : (o + 1) * OT], pso[o][:], AF.Copy
                    )
                sq = sqp.tile([P, H], dt.float32, name="sq")
                ssq = smp.tile([P, 1], dt.float32, name="ssq")
                nc.scalar.activation(sq[:], ot[:], AF.Square, accum_out=ssq[:])
                rms = smp.tile([P, 1], dt.float32, name="rms")
                nc.scalar.activation(
                    rms[:], ssq[:], AF.Sqrt, bias=epsb[:], scale=1.0 / H
                )
                rr = smp.tile([P, 1], dt.float32, name="rr")
                nc.vector.reciprocal(rr[:], rms[:])
                nc.vector.tensor_scalar_mul(ot[:], ot[:], rr[:])
                nc.vector.tensor_mul(ot[:], ot[:], nw[:])
                nc.sync.dma_start(out_d.ap()[sl * P : (sl + 1) * P, :], ot[:])

    nc.compile()
    return nc


# ---------------------------------------------------------------- host side
def _host_shared(w_attn, w_proj, norm_w):
    """Core-independent packed tensors."""
    f32 = np.float32

    def perm_halves(w):  # [H, n, HD] even/odd pairs -> halves
        return np.concatenate([w[..., 0::2], w[..., 1::2]], axis=-1)

    wq = perm_halves(w_attn[:, :H].reshape(H, NH, HD))
    wq = np.ascontiguousarray(
        wq.reshape(HT, P, NH, HD).transpose(2, 1, 0, 3)
    ).astype(BF16)
    wk = perm_halves(w_attn[:, H : H + KVC].reshape(H, NKV, HD))
    wk = np.ascontiguousarray(
        wk.reshape(HT, P, NKV, HD).transpose(2, 1, 0, 3)
    ).astype(BF16)
    wv = np.ascontiguousarray(
        w_attn[:, H + KVC :].reshape(HT, P, KVC)
    ).astype(BF16)
    wp = np.ascontiguousarray(w_proj.reshape(HT, P, H)).astype(BF16)

    jj, p, f = np.meshgrid(
        np.arange(JB), np.arange(P), np.arange(IT), indexing="ij"
    )
    # parity 0: query 2f vs key (128jj + p)
    mask0 = (2 * f >= 128 * jj + p).astype(BF16)
    # parity 1: query 2f+1 vs key (128jj + (p^1))  (pair-swapped x columns)
    mask1 = (2 * f + 1 >= 128 * jj + (p ^ 1)).astype(BF16)

    nw = np.ascontiguousarray(
        np.broadcast_to(norm_w.astype(f32), (P, H))
    )
    return wq, wk, wv, wp, (mask0, mask1), nw


def _cos_sin(pos):
    f32 = np.float32
    inv = 1.0 / (
        10000.0 ** (np.arange(0, HD, 2, dtype=f32) / f32(HD))
    )
    ang = inv[:, None].astype(f32) * pos[None, :].astype(f32)  # [HH, N]
    c, s = np.cos(ang).astype(BF16), np.sin(ang).astype(BF16)
    # duplicated across both partition halves (walrus wants equal base
    # partitions for SBUF tensor-tensor inputs)
    return (
        np.ascontiguousarray(np.concatenate([c, c], axis=0)),
        np.ascontiguousarray(np.concatenate([s, s], axis=0)),
    )


def make_in_maps(x, w_attn, w_proj, norm_w):
    x = np.asarray(x, dtype=np.float32)
    w_attn = np.asarray(w_attn, dtype=np.float32)
    w_proj = np.asarray(w_proj, dtype=np.float32)
    norm_w = np.asarray(norm_w, dtype=np.float32)

    wq, wk, wv, wp, (mask0, mask1), nw = _host_shared(w_attn, w_proj, norm_w)

    kc0, ks0 = _cos_sin(np.arange(S, dtype=np.float32))          # parity 0
    # parity 1: column j holds global row j^1 (pair-swapped x columns)
    kc1, ks1 = _cos_sin((np.arange(S) ^ 1).astype(np.float32))
    qc0, qs0 = _cos_sin(2.0 * np.arange(S_LOC, dtype=np.float32))
    qc1, qs1 = _cos_sin(2.0 * np.arange(S_LOC, dtype=np.float32) + 1.0)

    in_maps = []
    for c in range(N_CORES):
        b, par = c // 2, c % 2
        xt = x[b].T.astype(BF16)
        if par:
            xt = xt[:, np.arange(S) ^ 1]  # swap adjacent column pairs
        xt = np.ascontiguousarray(xt.reshape(HT, P, S))
        in_maps.append(
            {
                "xt": xt,
                "wq": wq,
                "wk": wk,
                "wv": wv,
                "wp": wp,
                "qcos": qc1 if par else qc0,
                "qsin": qs1 if par else qs0,
                "kcos": kc1 if par else kc0,
                "ksin": ks1 if par else ks0,
                "mask": mask1 if par else mask0,
                "nw": nw,
            }
        )
    return in_maps


def assemble_out(results):
    out = np.empty((B, S, H), dtype=np.float32)
    for c in range(N_CORES):
        b, par = c // 2, c % 2
        out[b, par::2, :] = results[c]["out"]
    return out


def kernel(x, w_attn, w_proj, norm_w):
    from concourse import bass_utils

    if "nc" not in _CACHE:
        _CACHE["nc"] = _build_nc()
    nc = _CACHE["nc"]

    in_maps = make_in_maps(x, w_attn, w_proj, norm_w)
    res = bass_utils.run_bass_kernel_spmd(
        nc, in_maps, core_ids=list(range(N_CORES))
    )
    return assemble_out(res.results)
